# revision 12
# baseline (speedup 1.0000x reference)
"""BinLinear (sign-quantized linear) Trainium2 kernel, mixed bf16 + fp8-DoubleRow.

Computes out = x @ sign(clip(w, -1, 1)).T for x[8192, 4096], w[4096, 4096],
data-parallel over 8 NeuronCores (each core takes 1024 rows of x and the full
weight matrix).

Sign weights are in {-1, 0, +1}: exact in every dtype, so the only numeric
error is quantizing x. The contraction is split: the first G=2304 channels run
as bf16-x (stationary) by fp8-sign (moving) matmuls, the remaining 1792 as
fp8e4 DoubleRow matmuls (256 contraction rows per instruction at 2x the bf16
MAC rate; e4m3 quantization of x there costs ~2.65% RMS on that slice).
Output error on the seed-0 data: rel_norm ~1.76e-2, absmax-rel ~1.46e-2
(< 2e-2 gate), deterministic.

Why the dtype choices: HW probing showed (a) fp8e4 DoubleRow streams 256
contraction x 512 output cols in 512 cycles (2x bf16, no +13% penalty),
(b) mixed bf16-stationary x fp8-moving is supported, (c) when all 8 cores
stream heavy DMA alongside DR matmuls, the chip power-caps the PE clock
2.4 -> 2.0 GHz. Shipping signs as fp8 (1B) and the output as fp16 (2B) cuts
DMA from ~49 to ~31 MB/core, which keeps the PE at 2.4 GHz.

Per-core plan (out_shard[1024, 4096], contraction 4096 = 18 bf16 k-tiles of
128 + 7 DoubleRow k-tiles of 256):
  - host precomputes sign(w), ships it fp8 pre-transposed/pre-tiled
    (pair-interleaved [*, ob, 2, 512] for the DR slice); x ships
    pre-transposed bf16 [2304, 1024] + e4m3 [7*128, 2, 1024], SBUF-resident.
  - ~40 dummy matmuls run during the initial DMA lead-in so the HAM clock
    gate reaches K=8/8 before real work (saves the ~3us cold ramp).
  - per output block of 512 columns, 8 PSUM banks (one per 128-row group)
    accumulate 18 bf16 + 7 DR matmuls; w tiles stream through SBUF once.
  - the final output block's w tiles are prefetched during the
    second-to-last block, and that block runs m-outer / k-inner from SBUF so
    its drains + output DMAs pipeline into the tail.
  - VectorE drains PSUM to fp16 SBUF tiles, DMA writes fp16 output, host
    upcasts to fp32.
"""

import numpy as np
import ml_dtypes

import concourse.mybir as mybir
import concourse.tile as tile
from concourse import bacc
from concourse.bass_utils import run_bass_kernel_spmd

N_CORES = 8
N_FULL, IN_CH, OUT_CH = 8192, 4096, 4096
P = 128
OBLK = 512           # output-channel columns per PSUM bank
G = 2304             # contraction channels computed in bf16
KB = G // P          # bf16 k-tiles (18)
KQ = (IN_CH - G) // (2 * P)  # fp8 DoubleRow k-tiles (7), 256 channels each
DR = mybir.MatmulPerfMode.DoubleRow


def build_nc(ns, out_ch):
    """Per-core SPMD program: out[ns, out_ch] = x @ sign(w).T, hybrid dtypes."""
    msub = ns // P        # PSUM banks in flight (8)
    nob = out_ch // OBLK  # output-channel blocks (8)
    assert msub <= 8

    nc = bacc.Bacc("TRN2", target_bir_lowering=False, debug=False)
    xb_d = nc.dram_tensor("xb", [G, ns], mybir.dt.bfloat16, kind="ExternalInput")
    xq_d = nc.dram_tensor("xq", [KQ * P, 2, ns], mybir.dt.float8e4,
                          kind="ExternalInput")
    # sign weights are {-1, 0, +1}: exact in fp8, so the bf16 sections run
    # mixed-dtype (bf16 stationary x, fp8 moving w) — halves w DMA traffic
    wb_d = nc.dram_tensor("wb", [G, out_ch], mybir.dt.float8e4,
                          kind="ExternalInput")
    wq_d = nc.dram_tensor("wq", [KQ * P, nob, 2, OBLK], mybir.dt.float8e4,
                          kind="ExternalInput")
    # output leaves the chip as fp16 (host upcasts): halves out DMA traffic
    # vs fp32 (power headroom), costs only ~2e-4 relative error (|out| < 500
    # so fp16 range is safe)
    out_d = nc.dram_tensor("out", [ns, out_ch], mybir.dt.float16,
                           kind="ExternalOutput")

    with tile.TileContext(nc) as tc:
        with (
            tc.tile_pool(name="xbp", bufs=1) as xbp,
            tc.tile_pool(name="xqp", bufs=1) as xqp,
            tc.tile_pool(name="wbp", bufs=26) as wbp,
            tc.tile_pool(name="wqp", bufs=10) as wqp,
            tc.tile_pool(name="wlp", bufs=1) as wlp,
            tc.tile_pool(name="opool", bufs=8) as opool,
            tc.tile_pool(name="pspool", bufs=1, space="PSUM") as pspool,
        ):
            # Dummy matmuls during the initial DMA lead-in (~10us) keep the
            # PE busy so the HAM clock-gate reaches K=8/8 before real work;
            # they scribble on bank ps_0, which ob 0's start=True then clears.
            warm = pspool.tile([P, OBLK], mybir.dt.float32, name="ps_0")
            wzero = wlp.tile([P, P], mybir.dt.bfloat16, name="wzero")
            nc.any.memset(wzero[:], 0.0)
            for _ in range(40):
                nc.tensor.matmul(
                    warm[:32, :P],
                    wzero[:, :32],
                    wzero[:],
                    start=True,
                    stop=True,
                )

            # x tiles are loaded lazily inside ob == 0's k-loop so the PE can
            # start as soon as the first (w, x) tile pair lands.
            xb_tiles = [None] * KB
            xq_tiles = [None] * KQ
            # final block's w tiles, prefetched during the second-to-last block
            wlb = [None] * KB
            wlq = [None] * KQ

            for ob in range(nob - 1):
                c0 = ob * OBLK
                psums = [
                    pspool.tile([P, OBLK], mybir.dt.float32, name=f"ps_{m}")
                    for m in range(msub)
                ]

                def dr_section(first):
                    for k in range(KQ):
                        if xq_tiles[k] is None:
                            xt = xqp.tile([P, 2, ns], mybir.dt.float8e4,
                                          name=f"xq_{k}")
                            nc.sync.dma_start(
                                out=xt[:], in_=xq_d[k * P:(k + 1) * P, :, :]
                            )
                            xq_tiles[k] = xt
                        wt = wqp.tile([P, 2, OBLK], mybir.dt.float8e4, name="wqt")
                        nc.sync.dma_start(
                            out=wt[:], in_=wq_d[k * P:(k + 1) * P, ob, :, :]
                        )
                        if ob == nob - 2:
                            wl = wlp.tile([P, 2, OBLK], mybir.dt.float8e4,
                                          name=f"wlq_{k}")
                            nc.gpsimd.dma_start(
                                out=wl[:],
                                in_=wq_d[k * P:(k + 1) * P, nob - 1, :, :],
                            )
                            wlq[k] = wl
                        for m in range(msub):
                            nc.tensor.matmul(
                                psums[m][:],
                                xq_tiles[k][:, :, m * P:(m + 1) * P],
                                wt[:],
                                start=(first and k == 0),
                                stop=(not first and k == KQ - 1),
                                perf_mode=DR,
                            )

                def bf16_section(first):
                    for k in range(KB):
                        if xb_tiles[k] is None:
                            xt = xbp.tile([P, ns], mybir.dt.bfloat16,
                                          name=f"xb_{k}")
                            nc.sync.dma_start(
                                out=xt[:], in_=xb_d[k * P:(k + 1) * P, :]
                            )
                            xb_tiles[k] = xt
                        wt = wbp.tile([P, OBLK], mybir.dt.float8e4, name="wt")
                        nc.sync.dma_start(
                            out=wt[:], in_=wb_d[k * P:(k + 1) * P, c0:c0 + OBLK]
                        )
                        if ob == nob - 2:
                            wl = wlp.tile([P, OBLK], mybir.dt.float8e4,
                                          name=f"wlb_{k}")
                            nc.gpsimd.dma_start(
                                out=wl[:],
                                in_=wb_d[k * P:(k + 1) * P, (nob - 1) * OBLK:],
                            )
                            wlb[k] = wl
                        for m in range(msub):
                            nc.tensor.matmul(
                                psums[m][:],
                                xb_tiles[k][:, m * P:(m + 1) * P],
                                wt[:],
                                start=(first and k == 0),
                                stop=(not first and k == KB - 1),
                            )

                if ob % 2 == 0:
                    dr_section(True)
                    bf16_section(False)
                else:
                    bf16_section(True)
                    dr_section(False)
                for m in range(msub):
                    ot = opool.tile([P, OBLK], mybir.dt.float16, name="ot")
                    nc.vector.tensor_copy(ot[:], psums[m][:])
                    nc.scalar.dma_start(
                        out=out_d[m * P:(m + 1) * P, c0:c0 + OBLK], in_=ot[:]
                    )

            c0 = (nob - 1) * OBLK
            psums = [
                pspool.tile([P, OBLK], mybir.dt.float32, name=f"ps_{m}")
                for m in range(msub)
            ]
            # phase 1: all bf16 chains (no mode switch vs the preceding
            # block, and none between m's); phase 2: all DR chains, each m
            # draining as soon as its accumulation completes
            for m in range(msub):
                for k in range(KB):
                    nc.tensor.matmul(
                        psums[m][:],
                        xb_tiles[k][:, m * P:(m + 1) * P],
                        wlb[k][:],
                        start=(k == 0),
                        stop=False,
                    )
            for m in range(msub):
                for k in range(KQ):
                    nc.tensor.matmul(
                        psums[m][:],
                        xq_tiles[k][:, :, m * P:(m + 1) * P],
                        wlq[k][:],
                        start=False,
                        stop=(k == KQ - 1),
                        perf_mode=DR,
                    )
                ot = opool.tile([P, OBLK], mybir.dt.float16, name="ot")
                nc.vector.tensor_copy(ot[:], psums[m][:])
                nc.scalar.dma_start(
                    out=out_d[m * P:(m + 1) * P, c0:c0 + OBLK], in_=ot[:]
                )
    nc.compile()
    return nc


def prep_in_maps(x, weights_real, n_cores=N_CORES):
    x = np.asarray(x, dtype=np.float32)
    weights_real = np.asarray(weights_real, dtype=np.float32)
    ns = x.shape[0] // n_cores
    nob = OUT_CH // OBLK

    sT = np.sign(np.clip(weights_real, -1.0, 1.0)).T  # [in, out]
    wb = np.ascontiguousarray(sT[:G]).astype(ml_dtypes.float8_e4m3)
    # DR slice: [(kk p), ob, 2, OBLK]; slot i of partition p in tile kk is
    # real contraction row G + 256*kk + 128*i + p
    wq = sT[G:].reshape(KQ, 2, P, nob, OBLK).transpose(0, 2, 3, 1, 4)
    wq = np.ascontiguousarray(wq.reshape(KQ * P, nob, 2, OBLK)).astype(
        ml_dtypes.float8_e4m3)

    xT = np.ascontiguousarray(x.T)  # [in, N]
    xb_full = xT[:G].astype(ml_dtypes.bfloat16)
    xq_full = xT[G:].reshape(KQ, 2, P, N_FULL).transpose(0, 2, 1, 3).reshape(
        KQ * P, 2, N_FULL).astype(ml_dtypes.float8_e4m3)

    return [
        {
            "xb": np.ascontiguousarray(xb_full[:, c * ns:(c + 1) * ns]),
            "xq": np.ascontiguousarray(xq_full[:, :, c * ns:(c + 1) * ns]),
            "wb": wb,
            "wq": wq,
        }
        for c in range(n_cores)
    ]


def run(x, weights_real, trace=False, **kwargs):
    nc = build_nc(N_FULL // N_CORES, OUT_CH)
    in_maps = prep_in_maps(x, weights_real)
    res = run_bass_kernel_spmd(nc, in_maps, list(range(N_CORES)), trace=trace,
                               **kwargs)
    out = np.concatenate(
        [np.asarray(res.results[c]["out"]).astype(np.float32)
         for c in range(N_CORES)], axis=0
    )
    return np.ascontiguousarray(out.astype(np.float32)), res


def kernel(x, weights_real):
    out, _ = run(x, weights_real)
    return out


# revision 13
# speedup vs baseline: 1.0154x; 1.0154x over previous
"""BinLinear (sign-quantized linear) Trainium2 kernel, mixed bf16 + fp8-DoubleRow.

Computes out = x @ sign(clip(w, -1, 1)).T for x[8192, 4096], w[4096, 4096],
data-parallel over 8 NeuronCores (each core takes 1024 rows of x and the full
weight matrix).

Sign weights are in {-1, 0, +1}: exact in every dtype, so the only numeric
error is quantizing x. The contraction is split: the first G=2304 channels run
as bf16-x (stationary) by fp8-sign (moving) matmuls, the remaining 1792 as
fp8e4 DoubleRow matmuls (256 contraction rows per instruction at 2x the bf16
MAC rate; e4m3 quantization of x there costs ~2.65% RMS on that slice).
Output error on the seed-0 data: rel_norm ~1.76e-2, absmax-rel ~1.46e-2
(< 2e-2 gate), deterministic.

Why the dtype choices: HW probing showed (a) fp8e4 DoubleRow streams 256
contraction x 512 output cols in 512 cycles (2x bf16, no +13% penalty),
(b) mixed bf16-stationary x fp8-moving is supported, (c) when all 8 cores
stream heavy DMA alongside DR matmuls, the chip power-caps the PE clock
2.4 -> 2.0 GHz. Shipping signs as fp8 (1B) and the output as fp16 (2B) cuts
DMA from ~49 to ~31 MB/core, which keeps the PE at 2.4 GHz.

Per-core plan (out_shard[1024, 4096], contraction 4096 = 18 bf16 k-tiles of
128 + 7 DoubleRow k-tiles of 256):
  - host precomputes sign(w), ships it fp8 pre-transposed/pre-tiled
    (pair-interleaved [*, ob, 2, 512] for the DR slice); x ships
    pre-transposed bf16 [2304, 1024] + e4m3 [7*128, 2, 1024], SBUF-resident.
  - ~40 dummy matmuls run during the initial DMA lead-in so the HAM clock
    gate reaches K=8/8 before real work (saves the ~3us cold ramp).
  - per output block of 512 columns, 8 PSUM banks (one per 128-row group)
    accumulate 18 bf16 + 7 DR matmuls; w tiles stream through SBUF once.
  - the final output block's w tiles are prefetched during the
    second-to-last block, and that block runs m-outer / k-inner from SBUF so
    its drains + output DMAs pipeline into the tail.
  - VectorE drains PSUM to fp16 SBUF tiles, DMA writes fp16 output, host
    upcasts to fp32.
"""

import numpy as np
import ml_dtypes

import concourse.mybir as mybir
import concourse.tile as tile
from concourse import bacc
from concourse.bass_utils import run_bass_kernel_spmd

N_CORES = 8
N_FULL, IN_CH, OUT_CH = 8192, 4096, 4096
P = 128
OBLK = 512           # output-channel columns per PSUM bank
G = 2304             # contraction channels computed in bf16
KB = G // P          # bf16 k-tiles (18)
KQ = (IN_CH - G) // (2 * P)  # fp8 DoubleRow k-tiles (7), 256 channels each
DR = mybir.MatmulPerfMode.DoubleRow


def build_nc(ns, out_ch):
    """Per-core SPMD program: out[ns, out_ch] = x @ sign(w).T, hybrid dtypes."""
    msub = ns // P        # PSUM banks in flight (8)
    nob = out_ch // OBLK  # output-channel blocks (8)
    assert msub <= 8

    nc = bacc.Bacc("TRN2", target_bir_lowering=False, debug=False)
    xb_d = nc.dram_tensor("xb", [G, ns], mybir.dt.bfloat16, kind="ExternalInput")
    xq_d = nc.dram_tensor("xq", [KQ * P, 2, ns], mybir.dt.float8e4,
                          kind="ExternalInput")
    # sign weights are {-1, 0, +1}: exact in fp8, so the bf16 sections run
    # mixed-dtype (bf16 stationary x, fp8 moving w) — halves w DMA traffic
    wb_d = nc.dram_tensor("wb", [G, out_ch], mybir.dt.float8e4,
                          kind="ExternalInput")
    wq_d = nc.dram_tensor("wq", [KQ * P, nob, 2, OBLK], mybir.dt.float8e4,
                          kind="ExternalInput")
    # output leaves the chip as fp16 (host upcasts): halves out DMA traffic
    # vs fp32 (power headroom), costs only ~2e-4 relative error (|out| < 500
    # so fp16 range is safe)
    out_d = nc.dram_tensor("out", [ns, out_ch], mybir.dt.float16,
                           kind="ExternalOutput")

    with tile.TileContext(nc) as tc:
        with (
            tc.tile_pool(name="xbp", bufs=1) as xbp,
            tc.tile_pool(name="xqp", bufs=1) as xqp,
            tc.tile_pool(name="wbp", bufs=16) as wbp,
            tc.tile_pool(name="wqp", bufs=8) as wqp,
            tc.tile_pool(name="wlp", bufs=1) as wlp,
            tc.tile_pool(name="opool", bufs=8) as opool,
            tc.tile_pool(name="pspool", bufs=1, space="PSUM") as pspool,
        ):
            # Dummy matmuls during the initial DMA lead-in (~10us) keep the
            # PE busy so the HAM clock-gate reaches K=8/8 before real work;
            # they scribble on bank ps_0, which ob 0's start=True then clears.
            warm = pspool.tile([P, OBLK], mybir.dt.float32, name="ps_0")
            wzero = wlp.tile([P, P], mybir.dt.bfloat16, name="wzero")
            nc.any.memset(wzero[:], 0.0)
            for _ in range(40):
                nc.tensor.matmul(
                    warm[:32, :P],
                    wzero[:, :32],
                    wzero[:],
                    start=True,
                    stop=True,
                )

            # x tiles are loaded lazily inside ob == 0's k-loop so the PE can
            # start as soon as the first (w, x) tile pair lands.
            xb_tiles = [None] * KB
            xq_tiles = [None] * KQ
            # final block's w tiles, prefetched during the second-to-last block
            wlb = [None] * KB
            wlq = [None] * KQ

            for ob in range(nob - 1):
                c0 = ob * OBLK
                psums = [
                    pspool.tile([P, OBLK], mybir.dt.float32, name=f"ps_{m}")
                    for m in range(msub)
                ]
                for k in range(KB):
                    wt = wbp.tile([P, OBLK], mybir.dt.float8e4, name="wt")
                    nc.sync.dma_start(
                        out=wt[:], in_=wb_d[k * P:(k + 1) * P, c0:c0 + OBLK]
                    )
                    if xb_tiles[k] is None:
                        xt = xbp.tile([P, ns], mybir.dt.bfloat16, name=f"xb_{k}")
                        nc.sync.dma_start(out=xt[:], in_=xb_d[k * P:(k + 1) * P, :])
                        xb_tiles[k] = xt
                    if ob == nob - 2:
                        wl = wlp.tile([P, OBLK], mybir.dt.float8e4, name=f"wlb_{k}")
                        nc.sync.dma_start(
                            out=wl[:],
                            in_=wb_d[k * P:(k + 1) * P, (nob - 1) * OBLK:],
                        )
                        wlb[k] = wl
                    for m in range(msub):
                        nc.tensor.matmul(
                            psums[m][:],
                            xb_tiles[k][:, m * P:(m + 1) * P],
                            wt[:],
                            start=(k == 0),
                            stop=False,
                        )
                for k in range(KQ):
                    wt = wqp.tile([P, 2, OBLK], mybir.dt.float8e4, name="wqt")
                    nc.sync.dma_start(
                        out=wt[:], in_=wq_d[k * P:(k + 1) * P, ob, :, :]
                    )
                    if xq_tiles[k] is None:
                        xt = xqp.tile([P, 2, ns], mybir.dt.float8e4, name=f"xq_{k}")
                        nc.sync.dma_start(out=xt[:], in_=xq_d[k * P:(k + 1) * P, :, :])
                        xq_tiles[k] = xt
                    if ob == nob - 2:
                        wl = wlp.tile([P, 2, OBLK], mybir.dt.float8e4,
                                      name=f"wlq_{k}")
                        nc.sync.dma_start(
                            out=wl[:], in_=wq_d[k * P:(k + 1) * P, nob - 1, :, :]
                        )
                        wlq[k] = wl
                    for m in range(msub):
                        nc.tensor.matmul(
                            psums[m][:],
                            xq_tiles[k][:, :, m * P:(m + 1) * P],
                            wt[:],
                            start=False,
                            stop=(k == KQ - 1),
                            perf_mode=DR,
                        )
                for m in range(msub):
                    ot = opool.tile([P, OBLK], mybir.dt.float16, name="ot")
                    nc.vector.tensor_copy(ot[:], psums[m][:])
                    nc.sync.dma_start(
                        out=out_d[m * P:(m + 1) * P, c0:c0 + OBLK], in_=ot[:]
                    )

            c0 = (nob - 1) * OBLK
            psums = [
                pspool.tile([P, OBLK], mybir.dt.float32, name=f"ps_{m}")
                for m in range(msub)
            ]
            for m in range(msub):
                for k in range(KB):
                    nc.tensor.matmul(
                        psums[m][:],
                        xb_tiles[k][:, m * P:(m + 1) * P],
                        wlb[k][:],
                        start=(k == 0),
                        stop=False,
                    )
                for k in range(KQ):
                    nc.tensor.matmul(
                        psums[m][:],
                        xq_tiles[k][:, :, m * P:(m + 1) * P],
                        wlq[k][:],
                        start=False,
                        stop=(k == KQ - 1),
                        perf_mode=DR,
                    )
                ot = opool.tile([P, OBLK], mybir.dt.float16, name="ot")
                nc.vector.tensor_copy(ot[:], psums[m][:])
                nc.sync.dma_start(
                    out=out_d[m * P:(m + 1) * P, c0:c0 + OBLK], in_=ot[:]
                )
    nc.compile()
    return nc


def prep_in_maps(x, weights_real, n_cores=N_CORES):
    x = np.asarray(x, dtype=np.float32)
    weights_real = np.asarray(weights_real, dtype=np.float32)
    ns = x.shape[0] // n_cores
    nob = OUT_CH // OBLK

    sT = np.sign(np.clip(weights_real, -1.0, 1.0)).T  # [in, out]
    wb = np.ascontiguousarray(sT[:G]).astype(ml_dtypes.float8_e4m3)
    # DR slice: [(kk p), ob, 2, OBLK]; slot i of partition p in tile kk is
    # real contraction row G + 256*kk + 128*i + p
    wq = sT[G:].reshape(KQ, 2, P, nob, OBLK).transpose(0, 2, 3, 1, 4)
    wq = np.ascontiguousarray(wq.reshape(KQ * P, nob, 2, OBLK)).astype(
        ml_dtypes.float8_e4m3)

    xT = np.ascontiguousarray(x.T)  # [in, N]
    xb_full = xT[:G].astype(ml_dtypes.bfloat16)
    xq_full = xT[G:].reshape(KQ, 2, P, N_FULL).transpose(0, 2, 1, 3).reshape(
        KQ * P, 2, N_FULL).astype(ml_dtypes.float8_e4m3)

    return [
        {
            "xb": np.ascontiguousarray(xb_full[:, c * ns:(c + 1) * ns]),
            "xq": np.ascontiguousarray(xq_full[:, :, c * ns:(c + 1) * ns]),
            "wb": wb,
            "wq": wq,
        }
        for c in range(n_cores)
    ]


def run(x, weights_real, trace=False, **kwargs):
    nc = build_nc(N_FULL // N_CORES, OUT_CH)
    in_maps = prep_in_maps(x, weights_real)
    res = run_bass_kernel_spmd(nc, in_maps, list(range(N_CORES)), trace=trace,
                               **kwargs)
    out = np.concatenate(
        [np.asarray(res.results[c]["out"]).astype(np.float32)
         for c in range(N_CORES)], axis=0
    )
    return np.ascontiguousarray(out.astype(np.float32)), res


def kernel(x, weights_real):
    out, _ = run(x, weights_real)
    return out
